# revision 2
# baseline (speedup 1.0000x reference)
# Triplet-margin loss kernel for Trainium2 (Bass/Tile), batch-sharded
# across 8 NeuronCores.
#
# reference math (torch F.pairwise_distance semantics):
#   d_ap[b,p] = || anc[b] - pos[b,p] + eps ||_2
#   d_an[b,n] = || anc[b] - neg[b,n] + eps ||_2
#   loss = mean_{b,p,n} max(d_ap[b,p] - d_an[b,n] + margin, 0)
#
# Per 128-row batch tile there are 24 distance columns ("slices"), each a
# [128, 1024] fp32 read. The kernel is HBM-DMA-bound (~25 MiB/core), so
# the slices are spread across three engine paths sized so every engine's
# busy time stays under the per-tile DMA time:
#   T1 (15 slices): dot a'.x on DVE (stt fp32, accum) + ||x||^2 on ACT
#       (Square, accum); d^2 = nrm - 2 dot + ||a'||^2.
#   B  (4 slices):  u = x - a' on GpSimd (bf16 out), sum u^2 on ACT.
#   C  (5 slices):  u = x - a' on GpSimd (bf16 out), sum u^2 on DVE as a
#       bf16 stt square (2x_1p perf mode), giving d^2 directly.
# The (p,n) pairing runs on ACT: relu(s_p - d_an) with per-partition bias
# s_p = d_ap[p] + margin and fp32 accumulate, 8 small ops per tile.
# Chunks feeding GpSimd are DMA'd first (it is the slowest per slice);
# each chunk is 4 slices = 2 MB with 16 KB/partition descriptors.
# Each core returns per-partition partial sums [128, 2]; the host sums
# and scales.

import numpy as np

import concourse.bacc as bacc
import concourse.mybir as mybir
import concourse.tile as tile
from concourse import bass_utils

B, Z = 2048, 1024
NUM_POS, NUM_NEG = 8, 16
NJ = NUM_POS + NUM_NEG
MARGIN, EPS = 1.0, 1e-6
N_CORES = 8
BL = B // N_CORES  # 256 rows of anc per core
P = 128
NT = BL // P  # 2 batch-tiles per core
CH = 4  # z-slices per DMA chunk
CHW = CH * Z
NCHUNK = NJ // CH  # 6 chunks per tile

# slice-type split by jj: [0, B_START) T1, [B_START, C_START) B, rest C
B_START = 15
C_START = 19
XP_BUFS = 8
# GpSimd-owned slices live in the high-jj chunks; DMA those first.
CHUNK_ORDER = list(range(NCHUNK - 1, -1, -1))

F32 = mybir.dt.float32
BF16 = mybir.dt.bfloat16
AF = mybir.ActivationFunctionType
OP = mybir.AluOpType


def _emit(tc, nc, anc, pos, neg, out):
    v = nc.vector
    act = nc.scalar
    gp = nc.gpsimd
    pos2 = pos.rearrange("(b j) z -> b (j z)", j=NUM_POS)  # [BL, 8*Z]
    neg2 = neg.rearrange("(b j) z -> b (j z)", j=NUM_NEG)  # [BL, 16*Z]
    with (
        tc.tile_pool(name="xp", bufs=XP_BUFS) as xp,
        tc.tile_pool(name="up", bufs=6) as up,
        tc.tile_pool(name="apool", bufs=2) as apool,
        tc.tile_pool(name="scp", bufs=1) as scp,
        tc.tile_pool(name="smp", bufs=2) as smp,
        tc.tile_pool(name="opool", bufs=1) as opool,
    ):
        osb = opool.tile([P, NT], F32, name="osb")
        dve_scr = scp.tile([P, Z], F32, name="dve_scr")
        act_scr = scp.tile([P, Z], BF16, name="act_scr")
        sq_scr = scp.tile([P, Z], BF16, name="sq_scr")
        ts_out = scp.tile([P, NUM_NEG], F32, name="ts_out")

        # prologue: both tiles' anc loads + a' = anc + eps + ||a'||^2
        ancs, aprimes, anrms = [], [], []
        for t in range(NT):
            b0 = t * P
            anc_in = apool.tile([P, Z], F32, name="anc_in")
            aprime = apool.tile([P, Z], F32, name="aprime")
            a_nrm = smp.tile([P, 1], F32, name="a_nrm")
            nc.sync.dma_start(anc_in[:, :], anc[b0 : b0 + P, :])
            v.tensor_scalar_add(aprime[:, :], anc_in[:, :], EPS)
            act.activation(
                act_scr[:, :], aprime[:, :], AF.Square, accum_out=a_nrm[:, 0:1]
            )
            ancs.append(anc_in)
            aprimes.append(aprime)
            anrms.append(a_nrm)

        for t in range(NT):
            b0 = t * P
            aprime = aprimes[t]
            a_nrm = anrms[t]
            dot = smp.tile([P, B_START], F32, name="dot")
            nrm = smp.tile([P, NJ], F32, name="nrm")
            d2c = smp.tile([P, B_START], F32, name="d2c")
            dt_ = smp.tile([P, NJ], F32, name="dt_")
            s_m = smp.tile([P, NUM_POS], F32, name="s_m")
            lp = smp.tile([P, NUM_POS], F32, name="lp")

            chunks = {}
            for c in CHUNK_ORDER:
                xt = xp.tile([P, CHW], F32, name="xt")
                if c < NUM_POS // CH:
                    src = pos2[b0 : b0 + P, c * CHW : (c + 1) * CHW]
                else:
                    cn = c - NUM_POS // CH
                    src = neg2[b0 : b0 + P, cn * CHW : (cn + 1) * CHW]
                nc.sync.dma_start(xt[:, :], src)
                chunks[c] = xt

            for c in CHUNK_ORDER:
                xt = chunks[c]
                for q in range(CH):
                    jj = c * CH + q
                    xs = xt[:, q * Z : (q + 1) * Z]
                    if jj < B_START:
                        v.scalar_tensor_tensor(
                            out=dve_scr[:, :],
                            in0=xs,
                            scalar=1.0,
                            in1=aprime[:, :],
                            op0=OP.bypass,
                            op1=OP.mult,
                            accum_out=dot[:, jj : jj + 1],
                        )
                        act.activation(
                            act_scr[:, :], xs, AF.Square, accum_out=nrm[:, jj : jj + 1]
                        )
                    else:
                        ut = up.tile([P, Z], BF16, name="ut")
                        gp.tensor_tensor(
                            out=ut[:, :], in0=xs, in1=aprime[:, :], op=OP.subtract
                        )
                        if jj < C_START:
                            act.activation(
                                act_scr[:, :],
                                ut[:, :],
                                AF.Square,
                                accum_out=nrm[:, jj : jj + 1],
                            )
                        else:
                            v.scalar_tensor_tensor(
                                out=sq_scr[:, :],
                                in0=ut[:, :],
                                scalar=1.0,
                                in1=ut[:, :],
                                op0=OP.bypass,
                                op1=OP.mult,
                                accum_out=nrm[:, jj : jj + 1],
                            )

            # T1 cols: d = sqrt((nrm - 2*dot) + ||a'||^2)
            v.scalar_tensor_tensor(
                out=d2c[:, :],
                in0=dot[:, :],
                scalar=-2.0,
                in1=nrm[:, 0:B_START],
                op0=OP.mult,
                op1=OP.add,
            )
            act.activation(
                dt_[:, 0:B_START], d2c[:, :], AF.Sqrt, bias=a_nrm[:, 0:1], scale=1.0
            )
            # sub cols already hold d^2 in nrm
            act.activation(dt_[:, B_START:NJ], nrm[:, B_START:NJ], AF.Sqrt)
            # s = d_ap + margin
            v.tensor_scalar_add(s_m[:, :], dt_[:, 0:NUM_POS], MARGIN)
            # lp[:,p] = sum_n relu(s_p - d_an)
            for p_i in range(NUM_POS):
                act.activation(
                    ts_out[:, :],
                    dt_[:, NUM_POS:NJ],
                    AF.Relu,
                    bias=s_m[:, p_i : p_i + 1],
                    scale=-1.0,
                    accum_out=lp[:, p_i : p_i + 1],
                )
            v.reduce_sum(osb[:, t : t + 1], lp[:, :], axis=mybir.AxisListType.X)
        nc.sync.dma_start(out[:, :], osb[:, :])


_NC_CACHE = None


def build():
    global _NC_CACHE
    if _NC_CACHE is None:
        nc = bacc.Bacc(
            "TRN2", target_bir_lowering=False, debug=False, num_devices=N_CORES
        )
        anc = nc.dram_tensor("anc", (BL, Z), F32, kind="ExternalInput").ap()
        pos = nc.dram_tensor("pos", (BL * NUM_POS, Z), F32, kind="ExternalInput").ap()
        neg = nc.dram_tensor("neg", (BL * NUM_NEG, Z), F32, kind="ExternalInput").ap()
        out = nc.dram_tensor("out", (P, NT), F32, kind="ExternalOutput").ap()
        with tile.TileContext(nc) as tc:
            _emit(tc, nc, anc, pos, neg, out)
        nc.compile()
        _NC_CACHE = nc
    return _NC_CACHE


def make_in_maps(anc_embedding, pos_embedding, neg_embedding):
    anc_embedding = np.asarray(anc_embedding, dtype=np.float32)
    pos_embedding = np.asarray(pos_embedding, dtype=np.float32)
    neg_embedding = np.asarray(neg_embedding, dtype=np.float32)
    in_maps = []
    for c in range(N_CORES):
        in_maps.append(
            {
                "anc": np.ascontiguousarray(anc_embedding[c * BL : (c + 1) * BL]),
                "pos": np.ascontiguousarray(
                    pos_embedding[c * BL * NUM_POS : (c + 1) * BL * NUM_POS]
                ),
                "neg": np.ascontiguousarray(
                    neg_embedding[c * BL * NUM_NEG : (c + 1) * BL * NUM_NEG]
                ),
            }
        )
    return in_maps


def combine(outs):
    # outs: list of [P, NT] per-core partial sums of relu(s - d_an)
    total = sum(o.astype(np.float64).sum() for o in outs)
    return np.float32(total / (B * NUM_POS * NUM_NEG))


def kernel(anc_embedding, pos_embedding, neg_embedding):
    nc = build()
    in_maps = make_in_maps(anc_embedding, pos_embedding, neg_embedding)
    res = bass_utils.run_bass_kernel_spmd(nc, in_maps, core_ids=list(range(N_CORES)))
    return combine([r["out"] for r in res.results])
